# revision 1
# baseline (speedup 1.0000x reference)
"""Trainium2 Bass kernel for nn_Activation2d (anti-aliased activation):
   y = downsample2d(leaky_relu(upsample2d(x)))  on x [8, 64, 256, 256] fp32.

Algorithm: both resamplers are separable 1D kaiser-sinc filters, expressed as
banded matrices baked with edge-replication clamping:
  A [512,256] = up matrix (includes ratio factor 2), B [256,512] = down.
  y = B_h . lrelu(A_h X A_w^T) . B_w^T

All four matmul passes use the "windowed-rhs" form (the banded filter matrix
is the rhs with its nonzero column window sliced), which both transposes the
data each pass (so the next contraction lands on the partition axis) and
streams the minimum number of PE columns:
  P1 (contract h):  lhsT = X slice   [h, w-blk]   rhs = A^T[h-blk, n-win]
                    -> U  [w, n]      2x2 matmuls, N=262, f32r (x bitcast)
  P2 (contract w):  lhsT = U slice   [w, n-blk]   rhs = A^T[w-blk, w'-win]
                    -> V  [n, w']     2x4 matmuls, N=~261, fp16
  lrelu fused into the PSUM->SBUF copy (ACT Prelu / DVE-Pool scalar_tensor_tensor)
  P3 (contract n):  lhsT = L slice   [n, w'-blk]  rhs = B^T[n-blk, mh-win]
                    -> D' [w', h'']   4x4 matmuls, N=~70, fp16
  P4 (contract w'): lhsT = D' slice  [w', h''-blk] rhs = B^T[w'-blk, mw-win]
                    -> y  [h'', w'']  4x2 matmuls, N=~70, fp16

PE cost/image ~4824 cycles (vs 6776 for the form-D alternation), ~2.0us.
Engine copies balanced across ACT/DVE/Pool so none exceeds the PE time.
Input is consumed directly as float32r via AP.bitcast (f32r is fp32 bits with
reduced-mantissa PE consumption) -- no cast pass.

Sharding: pure data parallel over batch -- core b computes x[b] [64,256,256].
"""
import math
from contextlib import ExitStack

import numpy as np

import concourse.bass as bass
import concourse.bacc as bacc
import concourse.tile as tile
import concourse.mybir as mybir
from concourse.bass_utils import run_bass_kernel_spmd

RATIO = 2
KSIZE = 12
SLOPE = 0.2
H = W = 256
NCORES = 8

F32R = mybir.dt.float32r
F16 = mybir.dt.float16
F32 = mybir.dt.float32


# ----------------------------------------------------------------------------
# filter construction (mirrors the reference's kaiser_sinc_filter1d)
# ----------------------------------------------------------------------------
def _kaiser_sinc_filter1d(cutoff, half_width, kernel_size):
    half_size = kernel_size // 2
    delta_f = 4.0 * half_width
    A = 2.285 * (half_size - 1) * math.pi * delta_f + 7.95
    if A > 50.0:
        beta = 0.1102 * (A - 8.7)
    elif A >= 21.0:
        beta = 0.5842 * (A - 21.0) ** 0.4 + 0.07886 * (A - 21.0)
    else:
        beta = 0.0
    window = np.kaiser(kernel_size, beta)
    if kernel_size % 2 == 0:
        time = np.arange(-half_size, half_size) + 0.5
    else:
        time = np.arange(kernel_size) - half_size
    filt = 2.0 * cutoff * window * np.sinc(2.0 * cutoff * time)
    filt = filt / filt.sum()
    return filt.astype(np.float32)


def build_A(n_in=H):
    f = _kaiser_sinc_filter1d(0.5 / RATIO, 0.6 / RATIO, KSIZE).astype(np.float64)
    A = np.zeros((2 * n_in, n_in), np.float64)
    for t in range(n_in):
        for j in range(6):
            A[2 * t, np.clip(t + j - 3, 0, n_in - 1)] += 2.0 * f[2 * j]
            A[2 * t + 1, np.clip(t + j - 2, 0, n_in - 1)] += 2.0 * f[2 * j + 1]
    return A.astype(np.float32)


def build_B(n_out=H):
    f = _kaiser_sinc_filter1d(0.5 / RATIO, 0.6 / RATIO, KSIZE).astype(np.float64)
    B = np.zeros((n_out, 2 * n_out), np.float64)
    for m in range(n_out):
        for k in range(KSIZE):
            B[m, np.clip(2 * m + k - 5, 0, 2 * n_out - 1)] += f[k]
    return B.astype(np.float32)


def _nz_cols(mat, even=False):
    nz = np.nonzero(np.any(mat != 0.0, axis=0))[0]
    lo, hi = int(nz[0]), int(nz[-1]) + 1
    if even:
        lo -= lo % 2
        hi += hi % 2
    return lo, hi


# ----------------------------------------------------------------------------
# bass program
# ----------------------------------------------------------------------------
def build_nc(n_img=64, repeats=1, in_batch=4,
             eng_u="vector", eng_d="vector", eng_y="scalar",
             lrelu_eng=("scalar", "scalar"),
             skew=(0, 1, 2, 3),
             psum_bufs=(1, 2, 2),
             sbuf_bufs=(2, 3, 6, 3, 3),
             drain_cols=None, big_v=False, big_d=False,
             first_batch=1, const_eng="pool", skew4=None):
    A = build_A()          # [512, 256]
    B = build_B()          # [256, 512]
    AT = A.T.copy()        # [256, 512] rows h (or w), cols n (or w')
    BT = B.T.copy()        # [512, 256] rows n (or w'), cols m

    # windows: per 128-row block of AT / BT, nonzero column range
    w_up_r = [_nz_cols(AT[128 * b:128 * (b + 1)], even=True) for b in range(2)]
    w_up16 = [_nz_cols(AT[128 * b:128 * (b + 1)]) for b in range(2)]
    w_dn = [_nz_cols(BT[128 * k:128 * (k + 1)]) for k in range(4)]

    nc = bacc.Bacc("TRN2", target_bir_lowering=False, debug=False,
                   num_devices=NCORES)
    x_ap = nc.dram_tensor("x", [n_img, H, W], F32, kind="ExternalInput").ap()
    y_ap = nc.dram_tensor("y", [n_img, H, W], F32, kind="ExternalOutput").ap()

    at32_dram = nc.inline_tensor(np.ascontiguousarray(AT), name="at32")
    at16_dram = nc.inline_tensor(np.ascontiguousarray(AT).astype(np.float16),
                                 name="at16")
    bt16_dram = nc.inline_tensor(np.ascontiguousarray(BT).astype(np.float16),
                                 name="bt16")

    def eng(name):
        return {"vector": nc.vector, "scalar": nc.scalar, "pool": nc.gpsimd}[name]

    def _shrink(dst, src):
        # diagnostic mode: preserve dependency structure, cut drain work
        if drain_cols is None:
            return dst, src
        return dst[:, 0:drain_cols], src[:, 0:drain_cols]

    def copy_op(engine_name, dst, src):
        dst, src = _shrink(dst, src)
        if engine_name == "scalar":
            nc.scalar.copy(dst, src)
        else:
            eng(engine_name).tensor_copy(dst, src)

    def lrelu_op(engine_name, dst, src):
        dst, src = _shrink(dst, src)
        if engine_name == "scalar":
            nc.scalar.activation(dst, src,
                                 mybir.ActivationFunctionType.Prelu,
                                 alpha=SLOPE)
        else:
            # (v * SLOPE) max v  in one fused pass
            eng(engine_name).scalar_tensor_tensor(
                dst, src, SLOPE, src,
                mybir.AluOpType.mult, mybir.AluOpType.max)

    with tile.TileContext(nc) as tc, ExitStack() as ctx:
        cpool = ctx.enter_context(tc.tile_pool(name="consts", bufs=1))
        xpool = ctx.enter_context(tc.tile_pool(name="xin", bufs=sbuf_bufs[0]))
        upool = ctx.enter_context(tc.tile_pool(name="u", bufs=sbuf_bufs[1]))
        lpool = ctx.enter_context(tc.tile_pool(name="l", bufs=sbuf_bufs[2]))
        dpool = ctx.enter_context(tc.tile_pool(name="d", bufs=sbuf_bufs[3]))
        opool = ctx.enter_context(tc.tile_pool(name="o", bufs=sbuf_bufs[4]))
        # PSUM budget (8 banks): pp1 1x[128,1024] (2 banks) + pp2 2x[128,1024]
        # (4 banks) + pp34 2x[128,512] (2 banks, shared by P3-out and P4-out)
        pp1 = ctx.enter_context(tc.tile_pool(name="pp1", bufs=psum_bufs[0], space="PSUM"))
        pp2 = ctx.enter_context(tc.tile_pool(name="pp2", bufs=psum_bufs[1], space="PSUM"))
        pp34 = ctx.enter_context(tc.tile_pool(name="pp34", bufs=psum_bufs[2], space="PSUM"))

        # ---- constants -------------------------------------------------
        cdma = {"pool": nc.gpsimd, "sync": nc.sync, "vector": nc.vector}[const_eng]
        AT32 = []   # P1 rhs, f32r (DMA'd fp32 bits; PE rounds on consumption)
        AT16 = []   # P2 rhs
        for b in range(2):
            t32 = cpool.tile([128, 512], F32R, tag=f"at32_{b}")
            cdma.dma_start(
                t32[:], at32_dram.ap()[128 * b:128 * (b + 1), :].bitcast(F32R))
            AT32.append(t32)
            t16 = cpool.tile([128, 512], F16, tag=f"at16_{b}")
            cdma.dma_start(t16[:], at16_dram.ap()[128 * b:128 * (b + 1), :])
            AT16.append(t16)
        BT16 = []   # P3/P4 rhs
        for k in range(4):
            t16 = cpool.tile([128, 256], F16, tag=f"bt16_{k}")
            cdma.dma_start(t16[:], bt16_dram.ap()[128 * k:128 * (k + 1), :])
            BT16.append(t16)

        # ---- per-image pipeline ----------------------------------------
        xr_tiles = {}  # c -> (tile, col offset)
        state = {}     # c -> dict with u / L / d aps
        img_seq = [i for _ in range(repeats) for i in range(n_img)]
        n = len(img_seq)
        # input DMA groups: a small first group shortens pipeline fill; groups
        # never cross the n_img boundary (c must stay contiguous)
        batch_starts = {}
        idx0 = 0
        first = True
        while idx0 < n:
            c0 = img_seq[idx0]
            nb = min(first_batch if first else in_batch, n - idx0, n_img - c0)
            batch_starts[idx0] = nb
            idx0 += nb
            first = False

        def stage1(idx):
            c = img_seq[idx]
            # -- input DMA: fp32, contiguous 1KB lines, batched ----------
            if idx in batch_starts:
                nb = batch_starts[idx]
                xf = xpool.tile([128, nb * 512], F32R, tag="xf")
                src = x_ap[c:c + nb].rearrange(
                    "c (b p) w -> p c b w", p=128).bitcast(F32R)
                nc.sync.dma_start(
                    xf[:].rearrange("p (c b w) -> p c b w", c=nb, b=2), src)
                for i in range(nb):
                    xr_tiles[idx + i] = (xf, 512 * i)
            xf, off = xr_tiles.pop(idx)

            # -- P1: U[w-blk wb, n] = sum_h X[h, w] A^T[h, n] ------------
            # one [128,1024] psum tile (2 banks), one-instruction drain
            ps1 = pp1.tile([128, 1024], F32, tag="ps1")
            for wb in range(2):
                for i, hb in enumerate(range(2)):
                    lo, hi = w_up_r[hb]
                    nc.tensor.matmul(
                        ps1[:, 512 * wb + lo:512 * wb + hi],
                        xf[:, off + 256 * hb + 128 * wb:
                           off + 256 * hb + 128 * (wb + 1)],
                        AT32[hb][:, lo:hi],
                        start=(i == 0), stop=(i == 1),
                        skip_group_check=True,
                    )
            u = upool.tile([128, 1024], F16, tag="u")
            copy_op(eng_u, u[:], ps1[:])
            state[idx] = {"u": u}

        def stage2(idx):
            # -- P2: V[n-blk ns, w'] = sum_w U[w, n] A^T[w, w'] ----------
            # two n-blocks packed per [128,1024] psum tile; lrelu fused in
            # the one-instruction drain
            u = state[idx]["u"]
            if big_v:
                ps = pp2.tile([128, 2048], F32, tag="ps2")
                for ns in range(4):
                    for i, b in enumerate(range(2)):
                        lo, hi = w_up16[b]
                        nc.tensor.matmul(
                            ps[:, 512 * ns + lo:512 * ns + hi],
                            u[:, 512 * b + 128 * ns: 512 * b + 128 * (ns + 1)],
                            AT16[b][:, lo:hi],
                            start=(i == 0), stop=(i == 1),
                            skip_group_check=True,
                        )
                l = lpool.tile([128, 2048], F16, tag="l")
                lrelu_op(lrelu_eng[0], l[:], ps[:])
                L = [l[:, 512 * k:512 * (k + 1)] for k in range(4)]
            else:
                L = []
                for pair in range(2):
                    ps = pp2.tile([128, 1024], F32, tag="ps2")
                    for half in range(2):
                        ns = 2 * pair + half
                        for i, b in enumerate(range(2)):
                            lo, hi = w_up16[b]
                            nc.tensor.matmul(
                                ps[:, 512 * half + lo:512 * half + hi],
                                u[:, 512 * b + 128 * ns: 512 * b + 128 * (ns + 1)],
                                AT16[b][:, lo:hi],
                                start=(i == 0), stop=(i == 1),
                                skip_group_check=True,
                            )
                    l = lpool.tile([128, 1024], F16, tag="l")
                    lrelu_op(lrelu_eng[pair], l[:], ps[:])
                    L.append(l[:, 0:512])
                    L.append(l[:, 512:1024])
            state[idx]["L"] = L

        def stage3(idx):
            # -- P3: D'[w'-blk j, mh] = sum_n L[n, w'] B^T[n, mh] --------
            L = state[idx].pop("L")
            if big_d:
                t3 = pp34.tile([128, 1024], F32, tag="ps34", name="ps3")
                tile_of = lambda j: t3
                col_of = lambda j: 256 * j
            else:
                tiles3 = [pp34.tile([128, 512], F32, tag="ps34", name="ps3a"),
                          pp34.tile([128, 512], F32, tag="ps34", name="ps3b")]
                tile_of = lambda j: tiles3[j // 2]
                col_of = lambda j: 256 * (j % 2)
            seen = set()
            for k in range(4):          # k outer: start as soon as L[k] ready
                lo, hi = w_dn[k]
                for j in range(4):
                    g = j // 2
                    nc.tensor.matmul(
                        tile_of(j)[:, col_of(j) + lo:col_of(j) + hi],
                        L[k][:, 128 * j:128 * (j + 1)],
                        BT16[k][:, lo:hi],
                        start=(g not in seen),
                        stop=(k == 3 and j >= 2 * g + 1),
                        skip_group_check=True,
                    )
                    seen.add(g)
            d = dpool.tile([128, 1024], F16, tag="d")
            if big_d:
                copy_op(eng_d, d[:], t3[:])
            else:
                for g in range(2):
                    copy_op(eng_d, d[:, 512 * g:512 * (g + 1)], tiles3[g][:])
            state[idx]["d"] = d

        def stage4(idx):
            # -- P4: y[mh-blk t, mw] = sum_w' D'[w', mh] B^T[w', mw] -----
            c = img_seq[idx]
            d = state[idx].pop("d")
            ps4 = pp34.tile([128, 512], F32, tag="ps34", name="ps4")
            first = True
            for j in range(4):          # j outer: start as soon as d[g] ready
                lo, hi = w_dn[j]
                for t in range(2):
                    nc.tensor.matmul(
                        ps4[:, 256 * t + lo:256 * t + hi],
                        d[:, 256 * j + 128 * t:256 * j + 128 * (t + 1)],
                        BT16[j][:, lo:hi],
                        start=first,
                        stop=(j == 3 and t == 1),
                        skip_group_check=True,
                    )
                    first = False
            o = opool.tile([128, 512], F32, tag="o")
            copy_op(eng_y, o[:], ps4[:])
            nc.sync.dma_start(
                y_ap[c].rearrange("(t p) w -> p t w", p=128),
                o[:].rearrange("p (t w) -> p t w", t=2))
            del state[idx]

        # software-pipelined emission
        s1, s2, s3, s4 = skew
        for s in range(n + max(skew)):
            if 0 <= s - s1 < n:
                stage1(s - s1)
            if 0 <= s - s3 < n:
                stage3(s - s3)
            if 0 <= s - s2 < n:
                stage2(s - s2)
            if 0 <= s - s4 < n:
                stage4(s - s4)

    nc.compile()
    return nc


_NC_CACHE = {}

# tuned configuration used by kernel()
BEST_CFG = dict()


def _get_nc(n_img, **overrides):
    cfg = dict(BEST_CFG, **overrides)
    key = (n_img, tuple(sorted((k, str(v)) for k, v in cfg.items())))
    if key not in _NC_CACHE:
        _NC_CACHE[key] = build_nc(n_img, **cfg)
    return _NC_CACHE[key]


def kernel(x: np.ndarray) -> np.ndarray:
    """x: [8, 64, 256, 256] fp32 -> y same shape."""
    x = np.asarray(x, dtype=np.float32)
    assert x.shape == (NCORES, 64, H, W), x.shape
    nc = _get_nc(64)
    in_maps = [{"x": x[b]} for b in range(NCORES)]
    res = run_bass_kernel_spmd(nc, in_maps, core_ids=list(range(NCORES)))
    return np.stack([res.results[b]["y"] for b in range(NCORES)], axis=0)



# revision 9
# speedup vs baseline: 2.3096x; 2.3096x over previous
"""Trainium2 Bass kernel for nn_Activation2d (anti-aliased activation):
   y = downsample2d(leaky_relu(upsample2d(x)))  on x [8, 64, 256, 256] fp32.

Algorithm: both resamplers are separable 1D kaiser-sinc filters, expressed as
banded matrices baked with edge-replication clamping:
  A [512,256] = up matrix (includes ratio factor 2), B [256,512] = down.
  y = B_h . lrelu(A_h X A_w^T) . B_w^T

All four matmul passes use the "windowed-rhs" form (the banded filter matrix
is the rhs with its nonzero column window sliced), which both transposes the
data each pass (so the next contraction lands on the partition axis) and
streams the minimum number of PE columns:
  P1 (contract h):  lhsT = X slice   [h, w-blk]   rhs = A^T[h-blk, n-win]
                    -> U  [w, n]      2x2 matmuls, N=262, f32r (x bitcast)
  P2 (contract w):  lhsT = U slice   [w, n-blk]   rhs = A^T[w-blk, w'-win]
                    -> V  [n, w']     2x4 matmuls, N=~261, fp16
  lrelu fused into the PSUM->SBUF copy (ACT Prelu / DVE-Pool scalar_tensor_tensor)
  P3 (contract n):  lhsT = L slice   [n, w'-blk]  rhs = B^T[n-blk, mh-win]
                    -> D' [w', h'']   4x4 matmuls, N=~70, fp16
  P4 (contract w'): lhsT = D' slice  [w', h''-blk] rhs = B^T[w'-blk, mw-win]
                    -> y  [h'', w'']  4x2 matmuls, N=~70, fp16

PE cost/image ~4824 cycles (vs 6776 for the form-D alternation), ~2.0us.
Input is consumed directly as float32r via AP.bitcast (f32r is fp32 bits with
reduced-mantissa PE consumption) -- no cast pass.

Steady-state engine budget per image (TimelineSim + HW large-R marginal):
  ACT: 2x Prelu [128,1024] + y copy [128,512]            ~2.69 us  <- bound
  DVE: u copy [128,1024] + 2x d copy [128,512]           ~2.51 us
  PE:  36 MMs, 4816 stream cycles + P3/P4 LDW pacing     ~2.4  us
All PSUM->SBUF drains are 1 elem/cycle/lane (PSUM fp32 src forbids DVE 2x
modes; GPSIMD and DMA have no PSUM port), so the 4608 drained columns/image
split across ACT+DVE set a ~2.65 us/img wall; measured marginal is ~2.85
us/img (~182 us per 64-image pass per core).  Rebalancing knobs (y_split,
v_split, big_v, big_d, engine swaps) were measured/simulated to move the
binding engine only upward -- the default assignment is the balance optimum
under the 8-bank PSUM budget, which blocks every larger-tile variant.

Sharding: pure data parallel over batch -- core b computes x[b] [64,256,256].
"""
import math
from contextlib import ExitStack

import numpy as np

import concourse.bass as bass
import concourse.bacc as bacc
import concourse.tile as tile
import concourse.mybir as mybir
from concourse.bass_utils import run_bass_kernel_spmd

RATIO = 2
KSIZE = 12
SLOPE = 0.2
H = W = 256
NCORES = 8

F32R = mybir.dt.float32r
F16 = mybir.dt.float16
F32 = mybir.dt.float32


# ----------------------------------------------------------------------------
# filter construction (mirrors the reference's kaiser_sinc_filter1d)
# ----------------------------------------------------------------------------
def _kaiser_sinc_filter1d(cutoff, half_width, kernel_size):
    half_size = kernel_size // 2
    delta_f = 4.0 * half_width
    A = 2.285 * (half_size - 1) * math.pi * delta_f + 7.95
    if A > 50.0:
        beta = 0.1102 * (A - 8.7)
    elif A >= 21.0:
        beta = 0.5842 * (A - 21.0) ** 0.4 + 0.07886 * (A - 21.0)
    else:
        beta = 0.0
    window = np.kaiser(kernel_size, beta)
    if kernel_size % 2 == 0:
        time = np.arange(-half_size, half_size) + 0.5
    else:
        time = np.arange(kernel_size) - half_size
    filt = 2.0 * cutoff * window * np.sinc(2.0 * cutoff * time)
    filt = filt / filt.sum()
    return filt.astype(np.float32)


def build_A(n_in=H):
    f = _kaiser_sinc_filter1d(0.5 / RATIO, 0.6 / RATIO, KSIZE).astype(np.float64)
    A = np.zeros((2 * n_in, n_in), np.float64)
    for t in range(n_in):
        for j in range(6):
            A[2 * t, np.clip(t + j - 3, 0, n_in - 1)] += 2.0 * f[2 * j]
            A[2 * t + 1, np.clip(t + j - 2, 0, n_in - 1)] += 2.0 * f[2 * j + 1]
    return A.astype(np.float32)


def build_B(n_out=H):
    f = _kaiser_sinc_filter1d(0.5 / RATIO, 0.6 / RATIO, KSIZE).astype(np.float64)
    B = np.zeros((n_out, 2 * n_out), np.float64)
    for m in range(n_out):
        for k in range(KSIZE):
            B[m, np.clip(2 * m + k - 5, 0, 2 * n_out - 1)] += f[k]
    return B.astype(np.float32)


def _nz_cols(mat, even=False):
    nz = np.nonzero(np.any(mat != 0.0, axis=0))[0]
    lo, hi = int(nz[0]), int(nz[-1]) + 1
    if even:
        lo -= lo % 2
        hi += hi % 2
    return lo, hi


# ----------------------------------------------------------------------------
# bass program
# ----------------------------------------------------------------------------
def build_nc(n_img=64, repeats=1, in_batch=4,
             eng_u="vector", eng_d="vector", eng_y="scalar",
             lrelu_eng=("scalar", "scalar"),
             skew=(0, 1, 2, 3),
             psum_bufs=(1, 2, 2),
             sbuf_bufs=(2, 3, 6, 3, 3),
             drain_cols=None, big_v=False, big_d=False,
             first_batch=1, const_eng="pool", skew4=None,
             y_split=None, v_split=None):
    A = build_A()          # [512, 256]
    B = build_B()          # [256, 512]
    AT = A.T.copy()        # [256, 512] rows h (or w), cols n (or w')
    BT = B.T.copy()        # [512, 256] rows n (or w'), cols m

    # windows: per 128-row block of AT / BT, nonzero column range
    w_up_r = [_nz_cols(AT[128 * b:128 * (b + 1)], even=True) for b in range(2)]
    w_up16 = [_nz_cols(AT[128 * b:128 * (b + 1)]) for b in range(2)]
    w_dn = [_nz_cols(BT[128 * k:128 * (k + 1)]) for k in range(4)]

    nc = bacc.Bacc("TRN2", target_bir_lowering=False, debug=False,
                   num_devices=NCORES)
    x_ap = nc.dram_tensor("x", [n_img, H, W], F32, kind="ExternalInput").ap()
    y_ap = nc.dram_tensor("y", [n_img, H, W], F32, kind="ExternalOutput").ap()

    at32_dram = nc.inline_tensor(np.ascontiguousarray(AT), name="at32")
    at16_dram = nc.inline_tensor(np.ascontiguousarray(AT).astype(np.float16),
                                 name="at16")
    bt16_dram = nc.inline_tensor(np.ascontiguousarray(BT).astype(np.float16),
                                 name="bt16")

    def eng(name):
        return {"vector": nc.vector, "scalar": nc.scalar, "pool": nc.gpsimd}[name]

    def _shrink(dst, src):
        # diagnostic mode: preserve dependency structure, cut drain work
        if drain_cols is None:
            return dst, src
        return dst[:, 0:drain_cols], src[:, 0:drain_cols]

    def copy_op(engine_name, dst, src):
        dst, src = _shrink(dst, src)
        if engine_name == "scalar":
            nc.scalar.copy(dst, src)
        else:
            eng(engine_name).tensor_copy(dst, src)

    def lrelu_op(engine_name, dst, src):
        dst, src = _shrink(dst, src)
        if engine_name == "scalar":
            nc.scalar.activation(dst, src,
                                 mybir.ActivationFunctionType.Prelu,
                                 alpha=SLOPE)
        else:
            # (v * SLOPE) max v  in one fused pass
            eng(engine_name).scalar_tensor_tensor(
                dst, src, SLOPE, src,
                mybir.AluOpType.mult, mybir.AluOpType.max)

    with tile.TileContext(nc) as tc, ExitStack() as ctx:
        cpool = ctx.enter_context(tc.tile_pool(name="consts", bufs=1))
        xpool = ctx.enter_context(tc.tile_pool(name="xin", bufs=sbuf_bufs[0]))
        upool = ctx.enter_context(tc.tile_pool(name="u", bufs=sbuf_bufs[1]))
        lpool = ctx.enter_context(tc.tile_pool(name="l", bufs=sbuf_bufs[2]))
        dpool = ctx.enter_context(tc.tile_pool(name="d", bufs=sbuf_bufs[3]))
        opool = ctx.enter_context(tc.tile_pool(name="o", bufs=sbuf_bufs[4]))
        # PSUM budget (8 banks): pp1 1x[128,1024] (2 banks) + pp2 2x[128,1024]
        # (4 banks) + pp34 2x[128,512] (2 banks, shared by P3-out and P4-out)
        pp1 = ctx.enter_context(tc.tile_pool(name="pp1", bufs=psum_bufs[0], space="PSUM"))
        pp2 = ctx.enter_context(tc.tile_pool(name="pp2", bufs=psum_bufs[1], space="PSUM"))
        pp34 = ctx.enter_context(tc.tile_pool(name="pp34", bufs=psum_bufs[2], space="PSUM"))

        # ---- constants -------------------------------------------------
        cdma = {"pool": nc.gpsimd, "sync": nc.sync, "vector": nc.vector}[const_eng]
        AT32 = []   # P1 rhs, f32r (DMA'd fp32 bits; PE rounds on consumption)
        AT16 = []   # P2 rhs
        for b in range(2):
            t32 = cpool.tile([128, 512], F32R, tag=f"at32_{b}")
            cdma.dma_start(
                t32[:], at32_dram.ap()[128 * b:128 * (b + 1), :].bitcast(F32R))
            AT32.append(t32)
            t16 = cpool.tile([128, 512], F16, tag=f"at16_{b}")
            cdma.dma_start(t16[:], at16_dram.ap()[128 * b:128 * (b + 1), :])
            AT16.append(t16)
        BT16 = []   # P3/P4 rhs
        for k in range(4):
            t16 = cpool.tile([128, 256], F16, tag=f"bt16_{k}")
            cdma.dma_start(t16[:], bt16_dram.ap()[128 * k:128 * (k + 1), :])
            BT16.append(t16)

        # ---- per-image pipeline ----------------------------------------
        xr_tiles = {}  # c -> (tile, col offset)
        state = {}     # c -> dict with u / L / d aps
        img_seq = [i for _ in range(repeats) for i in range(n_img)]
        n = len(img_seq)
        # input DMA groups: a small first group shortens pipeline fill; groups
        # never cross the n_img boundary (c must stay contiguous)
        batch_starts = {}
        idx0 = 0
        first = True
        while idx0 < n:
            c0 = img_seq[idx0]
            nb = min(first_batch if first else in_batch, n - idx0, n_img - c0)
            batch_starts[idx0] = nb
            idx0 += nb
            first = False

        def stage1(idx):
            c = img_seq[idx]
            # -- input DMA: fp32, contiguous 1KB lines, batched ----------
            if idx in batch_starts:
                nb = batch_starts[idx]
                xf = xpool.tile([128, nb * 512], F32R, tag="xf")
                src = x_ap[c:c + nb].rearrange(
                    "c (b p) w -> p c b w", p=128).bitcast(F32R)
                nc.sync.dma_start(
                    xf[:].rearrange("p (c b w) -> p c b w", c=nb, b=2), src)
                for i in range(nb):
                    xr_tiles[idx + i] = (xf, 512 * i)
            xf, off = xr_tiles.pop(idx)

            # -- P1: U[w-blk wb, n] = sum_h X[h, w] A^T[h, n] ------------
            # one [128,1024] psum tile (2 banks), one-instruction drain
            ps1 = pp1.tile([128, 1024], F32, tag="ps1")
            for wb in range(2):
                for i, hb in enumerate(range(2)):
                    lo, hi = w_up_r[hb]
                    nc.tensor.matmul(
                        ps1[:, 512 * wb + lo:512 * wb + hi],
                        xf[:, off + 256 * hb + 128 * wb:
                           off + 256 * hb + 128 * (wb + 1)],
                        AT32[hb][:, lo:hi],
                        start=(i == 0), stop=(i == 1),
                        skip_group_check=True,
                    )
            u = upool.tile([128, 1024], F16, tag="u")
            copy_op(eng_u, u[:], ps1[:])
            state[idx] = {"u": u}

        def stage2(idx):
            # -- P2: V[n-blk ns, w'] = sum_w U[w, n] A^T[w, w'] ----------
            # two n-blocks packed per [128,1024] psum tile; lrelu fused in
            # the one-instruction drain
            u = state[idx]["u"]
            if big_v:
                ps = pp2.tile([128, 2048], F32, tag="ps2")
                for ns in range(4):
                    for i, b in enumerate(range(2)):
                        lo, hi = w_up16[b]
                        nc.tensor.matmul(
                            ps[:, 512 * ns + lo:512 * ns + hi],
                            u[:, 512 * b + 128 * ns: 512 * b + 128 * (ns + 1)],
                            AT16[b][:, lo:hi],
                            start=(i == 0), stop=(i == 1),
                            skip_group_check=True,
                        )
                l = lpool.tile([128, 2048], F16, tag="l")
                lrelu_op(lrelu_eng[0], l[:], ps[:])
                L = [l[:, 512 * k:512 * (k + 1)] for k in range(4)]
            else:
                L = []
                for pair in range(2):
                    ps = pp2.tile([128, 1024], F32, tag="ps2")
                    for half in range(2):
                        ns = 2 * pair + half
                        for i, b in enumerate(range(2)):
                            lo, hi = w_up16[b]
                            nc.tensor.matmul(
                                ps[:, 512 * half + lo:512 * half + hi],
                                u[:, 512 * b + 128 * ns: 512 * b + 128 * (ns + 1)],
                                AT16[b][:, lo:hi],
                                start=(i == 0), stop=(i == 1),
                                skip_group_check=True,
                            )
                    l = lpool.tile([128, 1024], F16, tag="l")
                    if v_split is not None and pair == 1:
                        c = 1024 - v_split
                        lrelu_op(lrelu_eng[pair], l[:, 0:c], ps[:, 0:c])
                        lrelu_op("vector", l[:, c:1024], ps[:, c:1024])
                    else:
                        lrelu_op(lrelu_eng[pair], l[:], ps[:])
                    L.append(l[:, 0:512])
                    L.append(l[:, 512:1024])
            state[idx]["L"] = L

        def stage3(idx):
            # -- P3: D'[w'-blk j, mh] = sum_n L[n, w'] B^T[n, mh] --------
            L = state[idx].pop("L")
            if big_d:
                t3 = pp34.tile([128, 1024], F32, tag="ps34", name="ps3")
                tile_of = lambda j: t3
                col_of = lambda j: 256 * j
            else:
                tiles3 = [pp34.tile([128, 512], F32, tag="ps34", name="ps3a"),
                          pp34.tile([128, 512], F32, tag="ps34", name="ps3b")]
                tile_of = lambda j: tiles3[j // 2]
                col_of = lambda j: 256 * (j % 2)
            seen = set()
            for k in range(4):          # k outer: start as soon as L[k] ready
                lo, hi = w_dn[k]
                for j in range(4):
                    g = j // 2
                    nc.tensor.matmul(
                        tile_of(j)[:, col_of(j) + lo:col_of(j) + hi],
                        L[k][:, 128 * j:128 * (j + 1)],
                        BT16[k][:, lo:hi],
                        start=(g not in seen),
                        stop=(k == 3 and j >= 2 * g + 1),
                        skip_group_check=True,
                    )
                    seen.add(g)
            d = dpool.tile([128, 1024], F16, tag="d")
            if big_d:
                copy_op(eng_d, d[:], t3[:])
            else:
                for g in range(2):
                    copy_op(eng_d, d[:, 512 * g:512 * (g + 1)], tiles3[g][:])
            state[idx]["d"] = d

        def stage4(idx):
            # -- P4: y[mh-blk t, mw] = sum_w' D'[w', mh] B^T[w', mw] -----
            c = img_seq[idx]
            d = state[idx].pop("d")
            ps4 = pp34.tile([128, 512], F32, tag="ps34", name="ps4")
            first = True
            for j in range(4):          # j outer: start as soon as d[g] ready
                lo, hi = w_dn[j]
                for t in range(2):
                    nc.tensor.matmul(
                        ps4[:, 256 * t + lo:256 * t + hi],
                        d[:, 256 * j + 128 * t:256 * j + 128 * (t + 1)],
                        BT16[j][:, lo:hi],
                        start=first,
                        stop=(j == 3 and t == 1),
                        skip_group_check=True,
                    )
                    first = False
            o = opool.tile([128, 512], F32, tag="o")
            if y_split is not None:
                copy_op(eng_y, o[:, 0:y_split], ps4[:, 0:y_split])
                copy_op("vector", o[:, y_split:512], ps4[:, y_split:512])
            else:
                copy_op(eng_y, o[:], ps4[:])
            nc.sync.dma_start(
                y_ap[c].rearrange("(t p) w -> p t w", p=128),
                o[:].rearrange("p (t w) -> p t w", t=2))
            del state[idx]

        # software-pipelined emission
        s1, s2, s3, s4 = skew
        for s in range(n + max(skew)):
            if 0 <= s - s1 < n:
                stage1(s - s1)
            if 0 <= s - s3 < n:
                stage3(s - s3)
            if 0 <= s - s2 < n:
                stage2(s - s2)
            if 0 <= s - s4 < n:
                stage4(s - s4)

    nc.compile()
    return nc


_NC_CACHE = {}

# tuned configuration used by kernel()
BEST_CFG = dict()


def _get_nc(n_img, **overrides):
    cfg = dict(BEST_CFG, **overrides)
    key = (n_img, tuple(sorted((k, str(v)) for k, v in cfg.items())))
    if key not in _NC_CACHE:
        _NC_CACHE[key] = build_nc(n_img, **cfg)
    return _NC_CACHE[key]


def kernel(x: np.ndarray) -> np.ndarray:
    """x: [8, 64, 256, 256] fp32 -> y same shape."""
    x = np.asarray(x, dtype=np.float32)
    assert x.shape == (NCORES, 64, H, W), x.shape
    nc = _get_nc(64)
    in_maps = [{"x": x[b]} for b in range(NCORES)]
    res = run_bass_kernel_spmd(nc, in_maps, core_ids=list(range(NCORES)))
    return np.stack([res.results[b]["y"] for b in range(NCORES)], axis=0)



# revision 12
# speedup vs baseline: 2.4718x; 1.0702x over previous
"""Trainium2 Bass kernel for nn_Activation2d (anti-aliased activation):
   y = downsample2d(leaky_relu(upsample2d(x)))  on x [8, 64, 256, 256] fp32.

Algorithm: both resamplers are separable 1D kaiser-sinc filters, expressed as
banded matrices baked with edge-replication clamping:
  A [512,256] = up matrix (includes ratio factor 2), B [256,512] = down.
  y = B_h . lrelu(A_h X A_w^T) . B_w^T

All four matmul passes use the "windowed-rhs" form (the banded filter matrix
is the rhs with its nonzero column window sliced), which both transposes the
data each pass (so the next contraction lands on the partition axis) and
streams the minimum number of PE columns:
  P1 (contract h):  lhsT = X slice   [h, w-blk]   rhs = A^T[h-blk, n-win]
                    -> U  [w, n]      2x2 matmuls, N=262, f32r (x bitcast)
  P2 (contract w):  lhsT = U slice   [w, n-blk]   rhs = A^T[w-blk, w'-win]
                    -> V  [n, w']     2x4 matmuls, N=~261, fp16
  lrelu fused into the PSUM->SBUF copy (ACT Prelu / DVE-Pool scalar_tensor_tensor)
  P3 (contract n):  lhsT = L slice   [n, w'-blk]  rhs = B^T[n-blk, mh-win]
                    -> D' [w', h'']   4x4 matmuls, N=~70, fp16
  P4 (contract w'): lhsT = D' slice  [w', h''-blk] rhs = B^T[w'-blk, mw-win]
                    -> y  [h'', w'']  4x2 matmuls, N=~70, fp16

PE cost/image ~4824 cycles (vs 6776 for the form-D alternation), ~2.0us.
Input is consumed directly as float32r via AP.bitcast (f32r is fp32 bits with
reduced-mantissa PE consumption) -- no cast pass.

Steady-state engine budget per image (TimelineSim + HW large-R marginal):
  ACT: 2x Prelu [128,1024] + y copy [128,512]            ~2.69 us  <- bound
  DVE: u copy [128,1024] + 2x d copy [128,512]           ~2.51 us
  PE:  36 MMs, 4816 stream cycles + P3/P4 LDW pacing     ~2.4  us
All PSUM->SBUF drains are 1 elem/cycle/lane (PSUM fp32 src forbids DVE 2x
modes; GPSIMD and DMA have no PSUM port), so the 4608 drained columns/image
split across ACT+DVE set a ~2.65 us/img wall; measured marginal is ~2.85
us/img (~182 us per 64-image pass per core).  Rebalancing knobs (y_split,
v_split, big_v, big_d, engine swaps) were measured/simulated to move the
binding engine only upward -- the default assignment is the balance optimum
under the 8-bank PSUM budget, which blocks every larger-tile variant.

Sharding: pure data parallel over batch -- core b computes x[b] [64,256,256].
"""
import math
from contextlib import ExitStack

import numpy as np

import concourse.bass as bass
import concourse.bacc as bacc
import concourse.tile as tile
import concourse.mybir as mybir
from concourse.bass_utils import run_bass_kernel_spmd

RATIO = 2
KSIZE = 12
SLOPE = 0.2
H = W = 256
NCORES = 8

F32R = mybir.dt.float32r
F16 = mybir.dt.float16
F32 = mybir.dt.float32


# ----------------------------------------------------------------------------
# filter construction (mirrors the reference's kaiser_sinc_filter1d)
# ----------------------------------------------------------------------------
def _kaiser_sinc_filter1d(cutoff, half_width, kernel_size):
    half_size = kernel_size // 2
    delta_f = 4.0 * half_width
    A = 2.285 * (half_size - 1) * math.pi * delta_f + 7.95
    if A > 50.0:
        beta = 0.1102 * (A - 8.7)
    elif A >= 21.0:
        beta = 0.5842 * (A - 21.0) ** 0.4 + 0.07886 * (A - 21.0)
    else:
        beta = 0.0
    window = np.kaiser(kernel_size, beta)
    if kernel_size % 2 == 0:
        time = np.arange(-half_size, half_size) + 0.5
    else:
        time = np.arange(kernel_size) - half_size
    filt = 2.0 * cutoff * window * np.sinc(2.0 * cutoff * time)
    filt = filt / filt.sum()
    return filt.astype(np.float32)


def build_A(n_in=H):
    f = _kaiser_sinc_filter1d(0.5 / RATIO, 0.6 / RATIO, KSIZE).astype(np.float64)
    A = np.zeros((2 * n_in, n_in), np.float64)
    for t in range(n_in):
        for j in range(6):
            A[2 * t, np.clip(t + j - 3, 0, n_in - 1)] += 2.0 * f[2 * j]
            A[2 * t + 1, np.clip(t + j - 2, 0, n_in - 1)] += 2.0 * f[2 * j + 1]
    return A.astype(np.float32)


def build_B(n_out=H):
    f = _kaiser_sinc_filter1d(0.5 / RATIO, 0.6 / RATIO, KSIZE).astype(np.float64)
    B = np.zeros((n_out, 2 * n_out), np.float64)
    for m in range(n_out):
        for k in range(KSIZE):
            B[m, np.clip(2 * m + k - 5, 0, 2 * n_out - 1)] += f[k]
    return B.astype(np.float32)


def _nz_cols(mat, even=False):
    nz = np.nonzero(np.any(mat != 0.0, axis=0))[0]
    lo, hi = int(nz[0]), int(nz[-1]) + 1
    if even:
        lo -= lo % 2
        hi += hi % 2
    return lo, hi


# ----------------------------------------------------------------------------
# bass program
# ----------------------------------------------------------------------------
def build_nc(n_img=64, repeats=1, in_batch=4,
             eng_u="vector", eng_d="vector", eng_y="scalar",
             lrelu_eng=("scalar", "scalar"),
             skew=(0, 1, 2, 3),
             psum_bufs=(1, 2, 2),
             sbuf_bufs=(2, 3, 6, 3, 3),
             drain_cols=None, big_v=False, big_d=False,
             first_batch=1, const_eng="pool", skew4=None,
             y_split=None, v_split=None, order="1324"):
    A = build_A()          # [512, 256]
    B = build_B()          # [256, 512]
    AT = A.T.copy()        # [256, 512] rows h (or w), cols n (or w')
    BT = B.T.copy()        # [512, 256] rows n (or w'), cols m

    # windows: per 128-row block of AT / BT, nonzero column range
    w_up_r = [_nz_cols(AT[128 * b:128 * (b + 1)], even=True) for b in range(2)]
    w_up16 = [_nz_cols(AT[128 * b:128 * (b + 1)]) for b in range(2)]
    w_dn = [_nz_cols(BT[128 * k:128 * (k + 1)]) for k in range(4)]

    nc = bacc.Bacc("TRN2", target_bir_lowering=False, debug=False,
                   num_devices=NCORES)
    x_ap = nc.dram_tensor("x", [n_img, H, W], F32, kind="ExternalInput").ap()
    y_ap = nc.dram_tensor("y", [n_img, H, W], F32, kind="ExternalOutput").ap()

    at32_dram = nc.inline_tensor(np.ascontiguousarray(AT), name="at32")
    at16_dram = nc.inline_tensor(np.ascontiguousarray(AT).astype(np.float16),
                                 name="at16")
    bt16_dram = nc.inline_tensor(np.ascontiguousarray(BT).astype(np.float16),
                                 name="bt16")

    def eng(name):
        return {"vector": nc.vector, "scalar": nc.scalar, "pool": nc.gpsimd}[name]

    def _shrink(dst, src):
        # diagnostic mode: preserve dependency structure, cut drain work
        if drain_cols is None:
            return dst, src
        return dst[:, 0:drain_cols], src[:, 0:drain_cols]

    def copy_op(engine_name, dst, src):
        dst, src = _shrink(dst, src)
        if engine_name == "scalar":
            nc.scalar.copy(dst, src)
        else:
            eng(engine_name).tensor_copy(dst, src)

    def lrelu_op(engine_name, dst, src):
        dst, src = _shrink(dst, src)
        if engine_name == "scalar":
            nc.scalar.activation(dst, src,
                                 mybir.ActivationFunctionType.Prelu,
                                 alpha=SLOPE)
        else:
            # (v * SLOPE) max v  in one fused pass
            eng(engine_name).scalar_tensor_tensor(
                dst, src, SLOPE, src,
                mybir.AluOpType.mult, mybir.AluOpType.max)

    with tile.TileContext(nc) as tc, ExitStack() as ctx:
        cpool = ctx.enter_context(tc.tile_pool(name="consts", bufs=1))
        xpool = ctx.enter_context(tc.tile_pool(name="xin", bufs=sbuf_bufs[0]))
        upool = ctx.enter_context(tc.tile_pool(name="u", bufs=sbuf_bufs[1]))
        lpool = ctx.enter_context(tc.tile_pool(name="l", bufs=sbuf_bufs[2]))
        dpool = ctx.enter_context(tc.tile_pool(name="d", bufs=sbuf_bufs[3]))
        opool = ctx.enter_context(tc.tile_pool(name="o", bufs=sbuf_bufs[4]))
        # PSUM budget (8 banks): pp1 1x[128,1024] (2 banks) + pp2 2x[128,1024]
        # (4 banks) + pp34 2x[128,512] (2 banks, shared by P3-out and P4-out)
        pp1 = ctx.enter_context(tc.tile_pool(name="pp1", bufs=psum_bufs[0], space="PSUM"))
        pp2 = ctx.enter_context(tc.tile_pool(name="pp2", bufs=psum_bufs[1], space="PSUM"))
        pp34 = ctx.enter_context(tc.tile_pool(name="pp34", bufs=psum_bufs[2], space="PSUM"))

        # ---- constants -------------------------------------------------
        cdma = {"pool": nc.gpsimd, "sync": nc.sync, "vector": nc.vector}[const_eng]
        AT32 = []   # P1 rhs, f32r (DMA'd fp32 bits; PE rounds on consumption)
        AT16 = []   # P2 rhs
        for b in range(2):
            t32 = cpool.tile([128, 512], F32R, tag=f"at32_{b}")
            cdma.dma_start(
                t32[:], at32_dram.ap()[128 * b:128 * (b + 1), :].bitcast(F32R))
            AT32.append(t32)
            t16 = cpool.tile([128, 512], F16, tag=f"at16_{b}")
            cdma.dma_start(t16[:], at16_dram.ap()[128 * b:128 * (b + 1), :])
            AT16.append(t16)
        BT16 = []   # P3/P4 rhs
        for k in range(4):
            t16 = cpool.tile([128, 256], F16, tag=f"bt16_{k}")
            cdma.dma_start(t16[:], bt16_dram.ap()[128 * k:128 * (k + 1), :])
            BT16.append(t16)

        # ---- per-image pipeline ----------------------------------------
        xr_tiles = {}  # c -> (tile, col offset)
        state = {}     # c -> dict with u / L / d aps
        img_seq = [i for _ in range(repeats) for i in range(n_img)]
        n = len(img_seq)
        # input DMA groups: a small first group shortens pipeline fill; groups
        # never cross the n_img boundary (c must stay contiguous)
        batch_starts = {}
        idx0 = 0
        first = True
        while idx0 < n:
            c0 = img_seq[idx0]
            nb = min(first_batch if first else in_batch, n - idx0, n_img - c0)
            batch_starts[idx0] = nb
            idx0 += nb
            first = False

        def stage1(idx):
            c = img_seq[idx]
            # -- input DMA: fp32, contiguous 1KB lines, batched ----------
            if idx in batch_starts:
                nb = batch_starts[idx]
                xf = xpool.tile([128, nb * 512], F32R, tag="xf")
                src = x_ap[c:c + nb].rearrange(
                    "c (b p) w -> p c b w", p=128).bitcast(F32R)
                nc.sync.dma_start(
                    xf[:].rearrange("p (c b w) -> p c b w", c=nb, b=2), src)
                for i in range(nb):
                    xr_tiles[idx + i] = (xf, 512 * i)
            xf, off = xr_tiles.pop(idx)

            # -- P1: U[w-blk wb, n] = sum_h X[h, w] A^T[h, n] ------------
            # one [128,1024] psum tile (2 banks), one-instruction drain
            ps1 = pp1.tile([128, 1024], F32, tag="ps1")
            for wb in range(2):
                for i, hb in enumerate(range(2)):
                    lo, hi = w_up_r[hb]
                    nc.tensor.matmul(
                        ps1[:, 512 * wb + lo:512 * wb + hi],
                        xf[:, off + 256 * hb + 128 * wb:
                           off + 256 * hb + 128 * (wb + 1)],
                        AT32[hb][:, lo:hi],
                        start=(i == 0), stop=(i == 1),
                        skip_group_check=True,
                    )
            u = upool.tile([128, 1024], F16, tag="u")
            copy_op(eng_u, u[:], ps1[:])
            state[idx] = {"u": u}

        def stage2(idx):
            # -- P2: V[n-blk ns, w'] = sum_w U[w, n] A^T[w, w'] ----------
            # two n-blocks packed per [128,1024] psum tile; lrelu fused in
            # the one-instruction drain
            u = state[idx]["u"]
            if big_v:
                ps = pp2.tile([128, 2048], F32, tag="ps2")
                for ns in range(4):
                    for i, b in enumerate(range(2)):
                        lo, hi = w_up16[b]
                        nc.tensor.matmul(
                            ps[:, 512 * ns + lo:512 * ns + hi],
                            u[:, 512 * b + 128 * ns: 512 * b + 128 * (ns + 1)],
                            AT16[b][:, lo:hi],
                            start=(i == 0), stop=(i == 1),
                            skip_group_check=True,
                        )
                l = lpool.tile([128, 2048], F16, tag="l")
                lrelu_op(lrelu_eng[0], l[:], ps[:])
                L = [l[:, 512 * k:512 * (k + 1)] for k in range(4)]
            else:
                L = []
                for pair in range(2):
                    ps = pp2.tile([128, 1024], F32, tag="ps2")
                    for half in range(2):
                        ns = 2 * pair + half
                        for i, b in enumerate(range(2)):
                            lo, hi = w_up16[b]
                            nc.tensor.matmul(
                                ps[:, 512 * half + lo:512 * half + hi],
                                u[:, 512 * b + 128 * ns: 512 * b + 128 * (ns + 1)],
                                AT16[b][:, lo:hi],
                                start=(i == 0), stop=(i == 1),
                                skip_group_check=True,
                            )
                    l = lpool.tile([128, 1024], F16, tag="l")
                    if v_split is not None and pair == 1:
                        c = 1024 - v_split
                        lrelu_op(lrelu_eng[pair], l[:, 0:c], ps[:, 0:c])
                        lrelu_op("vector", l[:, c:1024], ps[:, c:1024])
                    else:
                        lrelu_op(lrelu_eng[pair], l[:], ps[:])
                    L.append(l[:, 0:512])
                    L.append(l[:, 512:1024])
            state[idx]["L"] = L

        def stage3(idx):
            # -- P3: D'[w'-blk j, mh] = sum_n L[n, w'] B^T[n, mh] --------
            L = state[idx].pop("L")
            if big_d:
                t3 = pp34.tile([128, 1024], F32, tag="ps34", name="ps3")
                tile_of = lambda j: t3
                col_of = lambda j: 256 * j
            else:
                tiles3 = [pp34.tile([128, 512], F32, tag="ps34", name="ps3a"),
                          pp34.tile([128, 512], F32, tag="ps34", name="ps3b")]
                tile_of = lambda j: tiles3[j // 2]
                col_of = lambda j: 256 * (j % 2)
            seen = set()
            for k in range(4):          # k outer: start as soon as L[k] ready
                lo, hi = w_dn[k]
                for j in range(4):
                    g = j // 2
                    nc.tensor.matmul(
                        tile_of(j)[:, col_of(j) + lo:col_of(j) + hi],
                        L[k][:, 128 * j:128 * (j + 1)],
                        BT16[k][:, lo:hi],
                        start=(g not in seen),
                        stop=(k == 3 and j >= 2 * g + 1),
                        skip_group_check=True,
                    )
                    seen.add(g)
            d = dpool.tile([128, 1024], F16, tag="d")
            if big_d:
                copy_op(eng_d, d[:], t3[:])
            else:
                for g in range(2):
                    copy_op(eng_d, d[:, 512 * g:512 * (g + 1)], tiles3[g][:])
            state[idx]["d"] = d

        def stage4(idx):
            # -- P4: y[mh-blk t, mw] = sum_w' D'[w', mh] B^T[w', mw] -----
            c = img_seq[idx]
            d = state[idx].pop("d")
            ps4 = pp34.tile([128, 512], F32, tag="ps34", name="ps4")
            first = True
            for j in range(4):          # j outer: start as soon as d[g] ready
                lo, hi = w_dn[j]
                for t in range(2):
                    nc.tensor.matmul(
                        ps4[:, 256 * t + lo:256 * t + hi],
                        d[:, 256 * j + 128 * t:256 * j + 128 * (t + 1)],
                        BT16[j][:, lo:hi],
                        start=first,
                        stop=(j == 3 and t == 1),
                        skip_group_check=True,
                    )
                    first = False
            o = opool.tile([128, 512], F32, tag="o")
            if y_split is not None:
                copy_op(eng_y, o[:, 0:y_split], ps4[:, 0:y_split])
                copy_op("vector", o[:, y_split:512], ps4[:, y_split:512])
            else:
                copy_op(eng_y, o[:], ps4[:])
            nc.sync.dma_start(
                y_ap[c].rearrange("(t p) w -> p t w", p=128),
                o[:].rearrange("p (t w) -> p t w", t=2))
            del state[idx]

        # software-pipelined emission
        s1, s2, s3, s4 = skew
        stages = {"1": (stage1, s1), "2": (stage2, s2),
                  "3": (stage3, s3), "4": (stage4, s4)}
        for s in range(n + max(skew)):
            for k in order:
                fn, sk = stages[k]
                if 0 <= s - sk < n:
                    fn(s - sk)

    nc.compile()
    return nc


_NC_CACHE = {}

# tuned configuration used by kernel(): emit P2 immediately after P1 so
# P2-pair0 (which gates ACT's lrelu drain) completes ~850ns earlier in each
# PE iteration, closing the measured ~160ns/image ACT idle bubble of the
# default 1-3-2-4 interleave.
BEST_CFG = dict(order="1234")


def _get_nc(n_img, **overrides):
    cfg = dict(BEST_CFG, **overrides)
    key = (n_img, tuple(sorted((k, str(v)) for k, v in cfg.items())))
    if key not in _NC_CACHE:
        _NC_CACHE[key] = build_nc(n_img, **cfg)
    return _NC_CACHE[key]


def kernel(x: np.ndarray) -> np.ndarray:
    """x: [8, 64, 256, 256] fp32 -> y same shape."""
    x = np.asarray(x, dtype=np.float32)
    assert x.shape == (NCORES, 64, H, W), x.shape
    nc = _get_nc(64)
    in_maps = [{"x": x[b]} for b in range(NCORES)]
    res = run_bass_kernel_spmd(nc, in_maps, core_ids=list(range(NCORES)))
    return np.stack([res.results[b]["y"] for b in range(NCORES)], axis=0)

